# revision 20
# baseline (speedup 1.0000x reference)
"""Trainium2 Bass kernel for AlignAttendPooling (M=1024 molecules, N=65536 nodes).

Strategy (hardcoded to the problem's input structure):
  - mol_node_matrix is one-hot with seg[i] = i % M (verified on host; generic
    numpy fallback otherwise), so the [M, N] matrices never touch the device.
  - Molecules sharded over 8 cores: core c owns molecules [128c, 128c+128).
    Node i = k*1024 + 128c + j belongs to local molecule j, chunk k (0..63).
  - fp16 everywhere on device (PE 1 cycle/row, DVE 2x mode needs packed
    2-byte contiguous operands - strided in0 kills it); f32 for softmax
    scores, PSUM accumulation and the final output. The mol state and W_hh
    path need fp16 precision (bf16 -> 2% error); the pooling path doesn't.
  - ctx = (sum_k wn_k * nf_k) @ W_att.T by softmax linearity (h never
    materialized); softmax normalization deferred to the pooled eviction.
  - mol0 = W_map.T @ (sum_k nf_k) + sum_k 0.99*relu(-x_k) via
    leaky(x) = x - 0.99*min(x, 0): linear part exact through host nfsum,
    relu part evicted in one tensor_scalar op per block.
  - Reductions over the 64 chunks: contiguous fp16 binary tree on DVE or
    PE-accumulated 512-column matmuls (identity weights), never
    tensor_reduce (1.3 ns/elem) or per-chunk 128-col matmuls.
  - Zero-bias program only (benchmark inputs); anything else falls back to
    a generic numpy implementation.
"""

import numpy as np

N, M, C, MD = 65536, 1024, 128, 256
NCORES = 8
MLOC = M // NCORES          # 128 molecules per core
K = N // M                  # 64 nodes per molecule (= chunks per core)
NEG = -1e9

_cache = {}


def _split_waits(nc, mybir, max_waits=1):
    """walrus in this container rejects >1 sync-wait on an instruction.
    Move excess waits onto preceding NOPs on the same engine."""
    n = 0
    for fn in nc.m.functions:
        for blk in fn.blocks:
            new_insts = []
            for inst in blk.instructions:
                si = inst.sync_info
                if si is not None and len(si.on_wait) > max_waits:
                    waits = list(si.on_wait)
                    excess, keep = waits[:-max_waits], waits[-max_waits:]
                    for i in range(0, len(excess), max_waits):
                        n += 1
                        new_insts.append(mybir.InstNoOp(
                            name=f"I-waitsplit-{n}",
                            engine=inst.engine,
                            ins=[], outs=[],
                            sync_info=mybir.SyncInfo(
                                on_wait=excess[i:i + max_waits], on_update=[]),
                        ))
                    inst.sync_info = mybir.SyncInfo(
                        on_wait=keep, on_update=list(si.on_update))
                new_insts.append(inst)
            blk.instructions = new_insts
    return n


def build_program():
    import concourse.bass as bass
    import concourse.mybir as mybir
    from concourse import tile

    AF = mybir.ActivationFunctionType
    ALU = mybir.AluOpType
    AX = mybir.AxisListType.X
    F32 = mybir.dt.float32
    F16 = mybir.dt.float16

    nc = bass.Bass('TRN2', target_bir_lowering=False, debug=False)

    def ap3(t_ap, dims):
        """AP with explicit free dims on top of a tile slice's partition dim."""
        return bass.AP(t_ap.tensor, t_ap.offset, [list(t_ap.ap[0])] + dims)

    # ---- I/O ----
    # wke cols (fp16): wmap.T [0:256) | ident [256:384) | W_att.T [384:512)
    #                  | w_al1 [512:514) | nfsum [514:642)
    wke = nc.dram_tensor('wke', [128, 642], F16, kind='ExternalInput')
    s2 = nc.dram_tensor('s2', [MLOC, K], F32, kind='ExternalInput')
    nfT = nc.dram_tensor('nfT', [C, K * MLOC], F16, kind='ExternalInput')
    # wkl cols (fp16): wih.T [0:768) | whh.T rows 0:128 [768:1536)
    #                  | whh.T rows 128:256 [1536:2304)
    wkl = nc.dram_tensor('wkl', [128, 2304], F16, kind='ExternalInput')
    nfR = nc.dram_tensor('nfR', [MLOC, K * C], F16, kind='ExternalInput')
    mol_out = nc.dram_tensor('mol_out', [MLOC, MD], F32, kind='ExternalOutput')

    HALF = K * MLOC         # 8192 free elements per md-half of mapped

    with tile.TileContext(nc) as tc, nc.allow_low_precision(reason='fp16 tree'):
        with tc.tile_pool(name='const', bufs=1) as const, \
             tc.tile_pool(name='big', bufs=1) as big, \
             tc.tile_pool(name='trp', bufs=1) as trp, \
             tc.tile_pool(name='rnd', bufs=2) as rnd, \
             tc.tile_pool(name='molp', bufs=3) as molp:

            # ---- DMAs (order = arrival order; one issue each) ----
            wke_sb = const.tile([128, 642], F16, name='wke_sb')
            nc.sync.dma_start(wke_sb[:], wke.ap())
            s2_sb = const.tile([MLOC, K], F32, name='s2_sb')
            nc.sync.dma_start(s2_sb[:], s2.ap())
            nfT_sb = big.tile([C, HALF], F16, name='nfT_sb')
            nc.sync.dma_start(nfT_sb[:, 0:4096], nfT.ap()[:, 0:4096])
            nc.sync.dma_start(nfT_sb[:, 4096:8192], nfT.ap()[:, 4096:8192])
            wkl_sb = const.tile([128, 2304], F16, name='wkl_sb')
            nc.sync.dma_start(wkl_sb[:], wkl.ap())
            nfR_sb = big.tile([MLOC, K * C], F16, name='nfR_sb')
            nc.sync.dma_start(nfR_sb[:, 0:4096], nfR.ap()[:, 0:4096])
            nc.sync.dma_start(nfR_sb[:, 4096:8192], nfR.ap()[:, 4096:8192])

            wmap_sb = wke_sb[:, 0:256]
            ident = wke_sb[:, 256:384]
            wattT = wke_sb[:, 384:512]
            wal1 = wke_sb[:, 512:514]
            nfsum_sb = wke_sb[:, 514:642]
            wih_sb = wkl_sb[:, 0:768]
            whh_sb = wkl_sb[:, 768:2304]

            # preload the exp table before round-1 softmax needs it
            dummy = const.tile([128, 1], F32, name='dummy')
            nc.scalar.activation(dummy[:], s2_sb[:, 0:1], AF.Exp)

            # ---- mol0 = W_map.T @ nfsum + sum_k 0.99*relu(-x_k) ----
            map_sb = big.tile([128, 2 * HALF], F16, name='map_sb')
            with tc.tile_pool(name='pmap', bufs=3, space='PSUM') as pmap, \
                 tc.tile_pool(name='pmol', bufs=1, space='PSUM') as pmol:
                psmol = pmol.tile([128, 384], F32, name='psmol')
                nc.tensor.matmul(psmol[:, 0:128], lhsT=wmap_sb[:, 0:128],
                                 rhs=nfsum_sb, start=True, stop=True)
                nc.tensor.matmul(psmol[:, 128:256], lhsT=wmap_sb[:, 128:256],
                                 rhs=nfsum_sb, start=True, stop=True)
                for g in range(2):
                    for bb in range(8):
                        ps = pmap.tile([128, 1024], F32, name='ps_map')
                        for h in range(2):
                            sl = slice((2 * bb + h) * 512, (2 * bb + h + 1) * 512)
                            nc.tensor.matmul(
                                ps[:, h * 512:(h + 1) * 512],
                                lhsT=wmap_sb[:, g * 128:(g + 1) * 128],
                                rhs=nfT_sb[:, sl], start=True, stop=True)
                        dst = map_sb[:, g * HALF + bb * 1024:
                                     g * HALF + (bb + 1) * 1024]
                        if bb % 2 == 0:
                            nc.scalar.activation(dst, ps[:], AF.Relu, scale=-0.99)
                        else:
                            nc.vector.tensor_scalar(
                                out=dst, in0=ps[:], scalar1=-0.99, scalar2=0.0,
                                op0=ALU.mult, op1=ALU.max)

                molT = molp.tile([128, MD], F16, name='molT')
                # md half 0: contiguous fp16 binary tree on DVE
                t1 = trp.tile([128, 4096], F16, name='t1')
                nc.vector.tensor_tensor(out=t1[:], in0=map_sb[:, 0:4096],
                                        in1=map_sb[:, 4096:8192], op=ALU.add)
                t2 = trp.tile([128, 2048], F16, name='t2')
                nc.vector.tensor_tensor(out=t2[:], in0=t1[:, 0:2048],
                                        in1=t1[:, 2048:4096], op=ALU.add)
                t3 = trp.tile([128, 1024], F16, name='t3')
                nc.vector.tensor_tensor(out=t3[:], in0=t2[:, 0:1024],
                                        in1=t2[:, 1024:2048], op=ALU.add)
                t4 = trp.tile([128, 512], F16, name='t4')
                nc.vector.tensor_tensor(out=t4[:], in0=t3[:, 0:512],
                                        in1=t3[:, 512:1024], op=ALU.add)
                t5 = trp.tile([128, 256], F16, name='t5')
                nc.vector.tensor_tensor(out=t5[:], in0=t4[:, 0:256],
                                        in1=t4[:, 256:512], op=ALU.add)
                tr0 = trp.tile([128, 128], F16, name='tr0')
                nc.vector.tensor_tensor(out=tr0[:], in0=t5[:, 0:128],
                                        in1=t5[:, 128:256], op=ALU.add)
                nc.vector.tensor_tensor(out=molT[:, 0:128], in0=tr0[:],
                                        in1=psmol[:, 0:128], op=ALU.add)
                # md half 1: PE column-group accumulation (16 x 512-col mms)
                ps_m1 = pmol.tile([128, 512], F32, name='ps_m1')
                for i in range(16):
                    sl = slice(HALF + i * 512, HALF + (i + 1) * 512)
                    nc.tensor.matmul(ps_m1[:], lhsT=ident, rhs=map_sb[:, sl],
                                     start=(i == 0), stop=(i == 15))
                em = trp.tile([128, 512], F16, name='em')
                nc.scalar.activation(em[:], ps_m1[:], AF.Copy)
                ea = trp.tile([128, 256], F16, name='ea')
                nc.vector.tensor_tensor(out=ea[:, 0:128], in0=em[:, 0:128],
                                        in1=em[:, 128:256], op=ALU.add)
                nc.vector.tensor_tensor(out=ea[:, 128:256], in0=em[:, 256:384],
                                        in1=em[:, 384:512], op=ALU.add)
                eb = trp.tile([128, 128], F16, name='eb')
                nc.vector.tensor_tensor(out=eb[:], in0=ea[:, 0:128],
                                        in1=ea[:, 128:256], op=ALU.add)
                nc.vector.tensor_tensor(out=molT[:, 128:256], in0=eb[:],
                                        in1=psmol[:, 128:256], op=ALU.add)

            # round-phase psum (after the precompute psum frees): 5 banks
            prnd_cm = tc.tile_pool(name='prnd', bufs=1, space='PSUM')
            prnd = prnd_cm.__enter__()
            pacc = prnd.tile([128, 512], F32, name='pacc')    # pool partials
            pmis = prnd.tile([128, 512], F32, name='pmis')    # mv | ctx
            ptf16 = prnd.tile([128, 512], F16, name='ptf16')  # transpose outs
            psA = prnd.tile([128, 512], F32, name='psA')      # r|z gates
            psB = prnd.tile([128, 512], F32, name='psB')      # i_n | h_n

            # molR = mol0 in [j, md] layout via PE transposes
            molR = molp.tile([128, MD], F16, name='molR')
            for g in range(2):
                mt_ps = ptf16[:, 128 + g * 128:256 + g * 128]
                nc.tensor.transpose(mt_ps, molT[:, g * 128:(g + 1) * 128], ident)
                nc.vector.tensor_copy(molR[:, g * 128:(g + 1) * 128], mt_ps)

            # ------------------------- rounds -------------------------
            sc_sb = big.tile([MLOC, K * C], F16, name='sc_sb')
            for r in range(2):
                # molvec + unnormalized softmax (normalization deferred)
                mv_ps = pmis[:, 0:1]
                nc.tensor.matmul(mv_ps, lhsT=molT[:, 0:128], rhs=wal1[:, 0:1],
                                 start=True, stop=False)
                nc.tensor.matmul(mv_ps, lhsT=molT[:, 128:256], rhs=wal1[:, 1:2],
                                 start=False, stop=True)
                lin = rnd.tile([128, K], F32, name='lin')
                nc.vector.tensor_scalar_add(lin[:], s2_sb[:], mv_ps)
                lin2 = rnd.tile([128, K], F32, name='lin2')
                nc.vector.tensor_scalar(out=lin2[:], in0=s2_sb[:], scalar1=mv_ps,
                                        scalar2=0.01, op0=ALU.add, op1=ALU.mult)
                a_t = rnd.tile([128, K], F32, name='a_t')
                nc.vector.tensor_tensor(out=a_t[:], in0=lin[:], in1=lin2[:],
                                        op=ALU.max)
                negmax = rnd.tile([128, 1], F32, name='negmax')
                nc.vector.tensor_reduce(negmax[:], a_t[:], axis=AX, op=ALU.max,
                                        negate=True)
                e_t = rnd.tile([128, K], F16, name='e_t')
                sumexp = rnd.tile([128, 1], F32, name='sumexp')
                nc.scalar.activation(e_t[:], a_t[:], AF.Exp, bias=negmax[:],
                                     accum_out=sumexp[:])
                rinv = rnd.tile([128, 1], F32, name='rinv')
                nc.vector.reciprocal(rinv[:], sumexp[:])
                # wne: e duplicated x2 so the scale's in1 has a packed last dim
                wne = rnd.tile([128, 2 * K], F16, name='wne')
                nc.vector.tensor_copy(ap3(wne[:, 0:128], [[2, K]]), e_t[:])
                nc.gpsimd.tensor_copy(ap3(wne[:, 1:128], [[2, K]]), e_t[:])

                # scale rows by e: 8 contiguous k-slices, in1 stride-0 middle
                for ks in range(8):
                    sl = slice(ks * 1024, (ks + 1) * 1024)
                    in0 = ap3(nfR_sb[:, sl], [[128, 8], [2, 64], [1, 2]])
                    outv = ap3(sc_sb[:, sl], [[128, 8], [2, 64], [1, 2]])
                    in1 = ap3(wne[:, ks * 16:(ks + 1) * 16],
                              [[2, 8], [0, 64], [1, 2]])
                    eng = nc.gpsimd if ks == 7 else nc.vector
                    eng.tensor_tensor(out=outv, in0=in0, in1=in1, op=ALU.mult)
                # pooled partials: PE column-group accumulation
                for i in range(16):
                    nc.tensor.matmul(pacc[:], lhsT=ident,
                                     rhs=sc_sb[:, i * 512:(i + 1) * 512],
                                     start=(i == 0), stop=(i == 15))
                # evict with 1/sumexp folded in, then 3 fp16 adds
                ep = rnd.tile([128, 512], F16, name='ep')
                nc.scalar.activation(ep[:], pacc[:], AF.Copy, scale=rinv[:])
                pa = rnd.tile([128, 256], F16, name='pa')
                nc.vector.tensor_tensor(out=pa[:, 0:128], in0=ep[:, 0:128],
                                        in1=ep[:, 128:256], op=ALU.add)
                nc.vector.tensor_tensor(out=pa[:, 128:256], in0=ep[:, 256:384],
                                        in1=ep[:, 384:512], op=ALU.add)
                pooled = rnd.tile([128, 128], F16, name='pooled')
                nc.vector.tensor_tensor(out=pooled[:], in0=pa[:, 0:128],
                                        in1=pa[:, 128:256], op=ALU.add)
                # ctxT = elu(W_att.T.T @ pooled.T) in [c', j] layout
                pT_ps = ptf16[:, 0:128]
                nc.tensor.transpose(pT_ps, pooled[:], ident)
                pT = rnd.tile([128, 128], F16, name='pT')
                nc.vector.tensor_copy(pT[:], pT_ps)
                ctx_ps = pmis[:, 128:256]
                nc.tensor.matmul(ctx_ps, lhsT=wattT, rhs=pT[:],
                                 start=True, stop=True)
                # elu(x) = (relu(x) - 1) + exp(min(x, 0))
                mneg = rnd.tile([128, 128], F16, name='mneg')
                nc.vector.tensor_scalar_min(mneg[:], ctx_ps, 0.0)
                e2 = rnd.tile([128, 128], F16, name='e2')
                nc.scalar.activation(e2[:], mneg[:], AF.Exp)
                rm1 = rnd.tile([128, 128], F16, name='rm1')
                nc.vector.tensor_scalar(out=rm1[:], in0=ctx_ps, scalar1=0.0,
                                        scalar2=-1.0, op0=ALU.max, op1=ALU.add)
                ctxT = rnd.tile([128, 128], F16, name='ctxT')
                nc.vector.tensor_tensor(out=ctxT[:], in0=rm1[:], in1=e2[:],
                                        op=ALU.add)

                # GRU gates in [j, gate] layout, 3 stationary sources
                psBi = psB[:, 0:256]
                psBh = psB[:, 256:512]
                nc.tensor.matmul(psA[:], lhsT=ctxT[:], rhs=wih_sb[:, 0:512],
                                 start=True, stop=False)
                nc.tensor.matmul(psBi, lhsT=ctxT[:], rhs=wih_sb[:, 512:768],
                                 start=True, stop=True)
                nc.tensor.matmul(psA[:], lhsT=molT[:, 0:128],
                                 rhs=whh_sb[:, 0:512], start=False, stop=False)
                nc.tensor.matmul(psBh, lhsT=molT[:, 0:128],
                                 rhs=whh_sb[:, 512:768], start=True, stop=False)
                nc.tensor.matmul(psA[:], lhsT=molT[:, 128:256],
                                 rhs=whh_sb[:, 768:1280], start=False, stop=True)
                nc.tensor.matmul(psBh, lhsT=molT[:, 128:256],
                                 rhs=whh_sb[:, 1280:1536], start=False, stop=True)
                # sigma(x) = 0.5 + 0.5*tanh(x/2); r-half first to unblock t1
                tz = rnd.tile([128, 512], F16, name='tz')
                nc.scalar.activation(tz[:, 0:256], psA[:, 0:256], AF.Tanh,
                                     scale=0.5)
                sig = rnd.tile([128, 512], F16, name='sig')
                nc.vector.tensor_scalar(out=sig[:, 0:256], in0=tz[:, 0:256],
                                        scalar1=0.5, scalar2=0.5,
                                        op0=ALU.mult, op1=ALU.add)
                nc.scalar.activation(tz[:, 256:512], psA[:, 256:512], AF.Tanh,
                                     scale=0.5)
                t1g = rnd.tile([128, 256], F16, name='t1g')
                nc.vector.tensor_tensor(out=t1g[:], in0=sig[:, 0:256],
                                        in1=psBh, op=ALU.mult)
                u1 = rnd.tile([128, 256], F32, name='u1')
                nc.vector.tensor_tensor(out=u1[:], in0=psBi, in1=t1g[:],
                                        op=ALU.add)
                nc.vector.tensor_scalar(out=sig[:, 256:512], in0=tz[:, 256:512],
                                        scalar1=0.5, scalar2=0.5,
                                        op0=ALU.mult, op1=ALU.add)
                # s1 = n*(1-z) + z*molR; z-terms computed while tanh(n) runs
                ta = rnd.tile([128, 256], F16, name='ta')
                nc.vector.tensor_tensor(out=ta[:], in0=sig[:, 256:512],
                                        in1=molR[:], op=ALU.mult)
                zc = rnd.tile([128, 256], F16, name='zc')
                nc.vector.tensor_scalar(out=zc[:], in0=sig[:, 256:512],
                                        scalar1=-1.0, scalar2=1.0,
                                        op0=ALU.mult, op1=ALU.add)
                n_t = rnd.tile([128, 256], F16, name='n_t')
                nc.scalar.activation(n_t[:], u1[:], AF.Tanh)
                tb_ = rnd.tile([128, 256], F16, name='tb_')
                nc.vector.tensor_tensor(out=tb_[:], in0=n_t[:], in1=zc[:],
                                        op=ALU.mult)
                s1 = rnd.tile([128, 256], F16, name='s1')
                nc.vector.tensor_tensor(out=s1[:], in0=ta[:], in1=tb_[:],
                                        op=ALU.add)
                if r == 0:
                    molR = molp.tile([128, MD], F16, name='molR')
                    nc.vector.tensor_scalar_max(molR[:], s1[:], 0.0)
                    molT = molp.tile([128, MD], F16, name='molT')
                    for g in range(2):
                        mt_ps = ptf16[:, 128 + g * 128:256 + g * 128]
                        nc.tensor.transpose(mt_ps,
                                            molR[:, g * 128:(g + 1) * 128],
                                            ident)
                        nc.vector.tensor_copy(molT[:, g * 128:(g + 1) * 128],
                                              mt_ps)
                else:
                    out_sb = rnd.tile([128, MD], F32, name='out_sb')
                    nc.vector.tensor_scalar_max(out_sb[:], s1[:], 0.0)
                    nc.sync.dma_start(mol_out.ap(), out_sb[:])

            prnd_cm.__exit__(None, None, None)

    import concourse.mybir as mybir2
    _split_waits(nc, mybir2, max_waits=1)
    return nc


def _prep_inputs(node_features, W_map, b_map, W_att, b_att, W_align, b_align,
                 W_ih, b_ih, W_hh, b_hh):
    f16, f32 = np.float16, np.float32
    nf = np.asarray(node_features, dtype=f32)
    s2_full = (nf.astype(np.float64)
               @ np.asarray(W_align, np.float64)[0, 256:384]).astype(f32)
    wmap = np.asarray(W_map, f32).T.astype(f16)
    ident = np.eye(128, dtype=f16)
    wattT = np.asarray(W_att, f32).T.astype(f16)
    wal1 = np.asarray(W_align, f32)[0, 0:256].reshape(2, 128).T.astype(f16)
    wih = np.asarray(W_ih, f32).T.astype(f16)
    whhT = np.asarray(W_hh, f32).T.astype(f16)         # [256, 768]
    wkl = np.ascontiguousarray(
        np.concatenate([wih, whhT[0:128], whhT[128:256]], axis=1))
    nf4 = nf.reshape(K, NCORES, MLOC, C)               # [k, core, j, c]
    s2r = s2_full.reshape(K, NCORES, MLOC)
    in_maps = []
    for c in range(NCORES):
        percore = nf4[:, c]                            # [k, j, c] f32
        nfT_c = np.ascontiguousarray(
            percore.transpose(2, 0, 1).reshape(C, K * MLOC).astype(f16))
        nfR_c = np.ascontiguousarray(
            percore.transpose(1, 0, 2).reshape(MLOC, K * C).astype(f16))
        nfsum_c = np.ascontiguousarray(
            percore.sum(axis=0).T.astype(f16))         # [c, j]
        wke = np.ascontiguousarray(
            np.concatenate([wmap, ident, wattT, wal1, nfsum_c], axis=1))
        s2_c = np.ascontiguousarray(s2r[:, c].T)       # [j, k] f32
        in_maps.append(dict(wke=wke, wkl=wkl, nfT=nfT_c, nfR=nfR_c, s2=s2_c))
    return in_maps


def _zero_bias(b_map, b_att, b_align, b_ih, b_hh):
    return not any(np.asarray(b).any()
                   for b in (b_map, b_att, b_align, b_ih, b_hh))


def _structure_ok(mol_node_matrix, mol_node_mask):
    mnm = np.asarray(mol_node_matrix)
    mask = np.asarray(mol_node_mask)
    if mnm.shape != (M, N) or mask.shape != (M, N):
        return False
    seg = np.arange(N) % M
    idx = np.arange(N)
    if not (mnm[seg, idx] == 1.0).all():
        return False
    if not (mask[seg, idx] == 0.0).all():
        return False
    if not np.array_equal(mnm.sum(axis=0), np.ones(N, dtype=mnm.dtype)):
        return False
    off = int((mask <= -1e8).sum())
    if off != M * N - N:
        return False
    return True


def _reference_fallback(node_features, mol_node_matrix, mol_node_mask,
                        W_map, b_map, W_att, b_att, W_align, b_align,
                        W_ih, b_ih, W_hh, b_hh):
    """Generic numpy implementation, used only if the one-hot structure
    check fails or biases are nonzero (never on the benchmark inputs)."""
    def leaky(x):
        return np.where(x > 0, x, 0.01 * x)
    nf = np.asarray(node_features, np.float32)
    mnm = np.asarray(mol_node_matrix, np.float32)
    msk = np.asarray(mol_node_mask, np.float32)
    mol = mnm @ leaky(nf @ W_map.T + b_map)
    for _ in range(2):
        h = nf @ W_att.T + b_att
        pooled = mnm.T @ mol
        a = leaky(np.concatenate([pooled, nf], -1) @ W_align.T + b_align)
        scores = mnm * a[:, 0][None, :] + msk
        z = scores - scores.max(1, keepdims=True)
        ez = np.exp(z)
        w = ez / ez.sum(1, keepdims=True)
        ctx = w @ h
        ctx = np.where(ctx > 0, ctx, np.exp(np.minimum(ctx, 0)) - 1)
        gi = ctx @ W_ih.T + b_ih
        gh = mol @ W_hh.T + b_hh
        i_r, i_z, i_n = np.split(gi, 3, -1)
        h_r, h_z, h_n = np.split(gh, 3, -1)
        r = 1 / (1 + np.exp(-(i_r + h_r)))
        zz = 1 / (1 + np.exp(-(i_z + h_z)))
        n = np.tanh(i_n + r * h_n)
        mol = np.maximum((1 - zz) * n + zz * mol, 0)
    return mol.astype(np.float32)


def run_on_device(in_maps):
    from concourse.bass_utils import run_bass_kernel_spmd
    if 'nc' not in _cache:
        _cache['nc'] = build_program()
    res = run_bass_kernel_spmd(_cache['nc'], in_maps, list(range(NCORES)))
    return res.results


def assemble(results):
    out = np.empty((M, MD), dtype=np.float32)
    for c in range(NCORES):
        out[c * MLOC:(c + 1) * MLOC, :] = results[c]['mol_out']
    return out


def kernel(node_features, mol_node_matrix, mol_node_mask,
           W_map, b_map, W_att, b_att, W_align, b_align,
           W_ih, b_ih, W_hh, b_hh):
    if not (_structure_ok(mol_node_matrix, mol_node_mask)
            and _zero_bias(b_map, b_att, b_align, b_ih, b_hh)):
        return _reference_fallback(
            node_features, mol_node_matrix, mol_node_mask, W_map, b_map,
            W_att, b_att, W_align, b_align, W_ih, b_ih, W_hh, b_hh)
    in_maps = _prep_inputs(node_features, W_map, b_map, W_att, b_att,
                           W_align, b_align, W_ih, b_ih, W_hh, b_hh)
    return assemble(run_on_device(in_maps))
